# revision 49
# baseline (speedup 1.0000x reference)
"""Trainium2 Bass kernel: multi-head attention (B=2, S=2048, H=768, 12 heads).

Returns (out, attn) like the reference. Sharding: 8 cores, each handles one
(batch, head-group-of-3) pair: core c -> batch c//4, heads 3*(c%4)..3*(c%4)+2.

Per-core plan (dual-orientation attention; every matmul contraction sits on
the partition dim, all operands fp32r for full-rate PE):
  - Host feeds transposed activations/weights; biases are folded in via an
    augmented ones-row (contraction padded to 896 = 7*128 chunks).
  - Projections: Q^T/K^T packed per head-pair, split into per-512-column
    tiles for fine-grained dependency release;
    V natural [2048, 192] (free dim padded to 256 for fp32r full rate).
  - A-side: scores[i,j] per 128-query chunk -> exp on ScalarE (accumulating
    the row-sum Z) -> 1/Z normalize on VectorE -> DMA the attn rows (the
    dominant 403MB output, fully contiguous 8KB rows).
  - B-side: scores^T[j,i] recomputed (cheaper than transposing 12.6M floats
    on-chip) -> exp -> E^T; per-j-chunk ctx deltas land in transient PSUM and
    VectorE accumulates them into SBUF (PSUM stays within 8 banks).
  - ctx normalized by 1/Z while leaving PSUM, transposed per 128x64 block on
    TensorE (transpose outputs must start at PSUM partition 0, hence per-head
    [64,3,128] ctx^T tiles + 3-way-accumulated output projection).
  - Emission order == per-engine execution order, so phases are interleaved
    by hand to keep ScalarE (the bottleneck: 2x 12.6M exps) at ~100% busy:
    head0-A pairs with the V projection, head(h)-A with head(h-1)-B.
  - Host gathers: attn concat, partial out summed across head-group cores.
Cost-model timeline: ~361us/core; engines ACT ~239us, DMA ~226us, PE ~225us
(balanced within 6%).
Startup is DMA-bound (x loads pipelined via 10 shared chunk slots); the
steady state holds ScalarE at ~100%; tail copies ride the then-idle ScalarE.
"""

import sys

sys.path.insert(0, "/opt/trn_rl_repo")

import numpy as np
from concourse import bass, masks, mybir, tile
from concourse.bass_utils import run_bass_kernel_spmd

F32 = mybir.dt.float32
F32R = mybir.dt.float32r
EXP = mybir.ActivationFunctionType.Exp

B, S, HID = 2, 2048, 768
NHEAD, D = 12, 64
NCORE = 8
HPC = 3            # heads per core
F = HPC * D        # 192 projected features per core
P = 128
SC = S // P        # 16 s-chunks
CCA = 7            # augmented contraction chunks (896 = 768 feat + bias row + pad)
CA = CCA * P       # 896
FP = 256           # V-projection free dim padded to 256 (fp32r full-rate needs >=256)
SCALE = 1.0 / float(np.sqrt(D))

ts = bass.ts

# toggles for test harness
TRACE = False
LAST_RESULTS = None


def split_sync_waits(nc, limit=1):
    """walrus codegen rejects CTRL instructions (the kernel-tail Drain)
    carrying more sem waits than it has slots; move excess waits onto
    preceding same-engine nops."""
    ctr = 0
    for f in nc.m.functions:
        for bb in f.blocks:
            out = []
            changed = False
            for inst in bb.instructions:
                si = inst.sync_info
                if si is not None and si.on_wait and len(si.on_wait) > limit:
                    waits = list(si.on_wait)
                    head, rest = waits[:limit], waits[limit:]
                    for i in range(0, len(rest), limit):
                        nop = mybir.InstNoOp(name=f"wsplit{ctr}", ins=[], outs=[])
                        ctr += 1
                        nop.engine = inst.engine
                        nop.sync_info = mybir.SyncInfo(
                            on_wait=rest[i : i + limit], on_update=[]
                        )
                        out.append(nop)
                    si.on_wait = head
                    changed = True
                out.append(inst)
            if changed:
                bb.instructions = out
    return ctr


def build_program():
    nc = bass.Bass()

    xqT = nc.declare_dram_parameter("xqT", [CA, S], F32R, isOutput=False)
    xkT = nc.declare_dram_parameter("xkT", [CA, S], F32R, isOutput=False)
    xvT = nc.declare_dram_parameter("xvT", [CA, S], F32R, isOutput=False)
    wqT = nc.declare_dram_parameter("wqT", [CA, F], F32R, isOutput=False)
    wkT = nc.declare_dram_parameter("wkT", [CA, F], F32R, isOutput=False)
    wvT = nc.declare_dram_parameter("wvT", [CA, FP], F32R, isOutput=False)
    woT = nc.declare_dram_parameter("woT", [F, HID], F32R, isOutput=False)
    attn_o = nc.declare_dram_parameter("attn", [HPC, S, S], F32, isOutput=True)
    out_o = nc.declare_dram_parameter("outp", [S, HID], F32, isOutput=True)

    from contextlib import ExitStack

    with tile.TileContext(nc) as tc:
        with ExitStack() as ctx:
            wpool = ctx.enter_context(tc.tile_pool(name="w", bufs=1))
            qkpool = ctx.enter_context(tc.tile_pool(name="qk", bufs=1))
            vpool = ctx.enter_context(tc.tile_pool(name="v", bufs=1))
            epool = ctx.enter_context(tc.tile_pool(name="e", bufs=2))
            etpool = ctx.enter_context(tc.tile_pool(name="et", bufs=2))
            stat = ctx.enter_context(tc.tile_pool(name="st", bufs=4))
            zpool = ctx.enter_context(tc.tile_pool(name="z", bufs=34))
            ctpool = ctx.enter_context(tc.tile_pool(name="ct", bufs=4))
            accpool = ctx.enter_context(tc.tile_pool(name="acc", bufs=2))
            opool = ctx.enter_context(tc.tile_pool(name="o", bufs=2))
            # PSUM (8 banks): psA slot [128,1024] x2 bufs = 4 banks (scores-A,
            # QK-proj, even outproj), psB slot [128,1024] x2 bufs = 4 banks
            # (scores-B, V-proj, ctx deltas, transposes, odd outproj).
            psA = ctx.enter_context(tc.tile_pool(name="psA", bufs=2, space="PSUM"))
            psB = ctx.enter_context(tc.tile_pool(name="psB", bufs=2, space="PSUM"))

            ident = wpool.tile([P, P], F32, tag="ident")
            masks.make_identity(nc, ident[:])

            # ---- weights ----
            # x chunks + projection weight chunks live in a scoped pool that
            # closes after phase 1, freeing ~72KB/partition for the ctx pool
            xstack = ExitStack()
            xpool = xstack.enter_context(tc.tile_pool(name="x", bufs=CCA + 3))
            # one batched DMA per weight tensor (24 small DMAs would each pay
            # the fixed per-transfer overhead before the x loads can start)
            wq_a = xpool.tile([P, CCA, F], F32R, tag="wqa", bufs=1)
            wk_a = xpool.tile([P, CCA, F], F32R, tag="wka", bufs=1)
            wv_a = xpool.tile([P, CCA, FP], F32R, tag="wva", bufs=1)
            nc.sync.dma_start(wq_a[:], wqT.rearrange("(c p) f -> p c f", p=P))
            nc.sync.dma_start(wk_a[:], wkT.rearrange("(c p) f -> p c f", p=P))
            nc.sync.dma_start(wv_a[:], wvT.rearrange("(c p) f -> p c f", p=P))
            wq_c = [wq_a[:, c, :] for c in range(CCA)]
            wk_c = [wk_a[:, c, :] for c in range(CCA)]
            wv_c = [wv_a[:, c, :] for c in range(CCA)]
            wo_a = wpool.tile([D, HPC, HID], F32R, tag="woa")
            nc.sync.dma_start(wo_a[:], woT.rearrange("(h d) o -> d h o", d=D))
            wo_h = [wo_a[:, h, :] for h in range(HPC)]

            # ---- Q/K projections ----
            # per-512-column tiles: finer dependency granularity lets the
            # first exps start as soon as their column group is projected
            qtn = [qkpool.tile([P, 512], F32R, tag=f"qtn{n}", name=f"qtn{n}") for n in range(4)]
            qbn = [qkpool.tile([D, 512], F32R, tag=f"qbn{n}", name=f"qbn{n}") for n in range(4)]
            ktn = [qkpool.tile([P, 512], F32R, tag=f"ktn{n}", name=f"ktn{n}") for n in range(4)]
            kbn = [qkpool.tile([D, 512], F32R, tag=f"kbn{n}", name=f"kbn{n}") for n in range(4)]
            v_sb = [vpool.tile([P, F], F32R, tag=f"v{s}", name=f"v{s}") for s in range(SC)]

            def _grp(ta, tb, h, g):
                t = ta[g] if h < 2 else tb[g]
                r0 = D * (h % 2) if h < 2 else 0
                return t[r0 : r0 + D, :]

            def _sel(ta, tb, h, c):
                t = ta[c // 4] if h < 2 else tb[c // 4]
                r0 = D * (h % 2) if h < 2 else 0
                o = (c % 4) * P
                return t[r0 : r0 + D, o : o + P]

            def qgrp(h, g):
                return _grp(qtn, qbn, h, g)

            def qsel(h, c):
                return _sel(qtn, qbn, h, c)

            def kgrp(h, g):
                return _grp(ktn, kbn, h, g)

            def ksel(h, c):
                return _sel(ktn, kbn, h, c)

            for xdram, w_c, dst01, dst2 in (
                (xqT, wq_c, qtn, qbn),
                (xkT, wk_c, ktn, kbn),
            ):
                xc = [xpool.tile([P, S], F32R, tag="xc", name="xc") for _ in range(CCA)]
                for c in range(CCA):
                    nc.sync.dma_start(xc[c][:], xdram[ts(c, P), :])
                for m in range(2):  # f rows 0:128 / 128:192
                    mp = P if m == 0 else F - P
                    for n in range(4):
                        ps = psB.tile([P, 512], F32, tag="psB", name="pj")
                        for c in range(CCA):
                            nc.tensor.matmul(
                                ps[:mp, :],
                                (w_c[c][:, m * P : m * P + mp]),
                                (xc[c][:, ts(n, 512)]),
                                start=(c == 0),
                                stop=(c == CCA - 1),
                            )
                        if m == 0:
                            nc.vector.tensor_copy(dst01[n][:], ps[:])
                        else:
                            nc.vector.tensor_copy(dst2[n][:], ps[0:D, :])

            xcv = [xpool.tile([P, S], F32R, tag="xc", name="xc") for _ in range(CCA)]

            def v_proj(s):
                ps = psB.tile([P, 512], F32, tag="psB", name="pv")
                for c in range(CCA):
                    nc.tensor.matmul(
                        ps[:, :FP],
                        (xcv[c][:, ts(s, P)]),
                        (wv_c[c][:]),
                        start=(c == 0),
                        stop=(c == CCA - 1),
                    )
                nc.vector.tensor_copy(v_sb[s][:], ps[:, :F])

            # ---- attention ----
            zinv = {}
            def a_side(h, ic):
                # scores[i,j] for one 128-query chunk; softmax row pass
                e_t = epool.tile([P, S], F32, tag="e", name="e")
                zp = stat.tile([P, 2], F32, tag="zp", name="zp")
                for jh in range(2):
                    ps = psA.tile([P, 1024], F32, tag="psA", name="pa")
                    for g in range(2):
                        nc.tensor.matmul(
                            ps[:, ts(g, 512)],
                            (qsel(h, ic)),
                            (kgrp(h, jh * 2 + g)),
                            start=True,
                            stop=True,
                        )
                    nc.scalar.activation(
                        e_t[:, ts(jh, 1024)], ps[:], EXP,
                        scale=SCALE, accum_out=zp[:, jh : jh + 1],
                    )
                zs = stat.tile([P, 1], F32, tag="zs", name="zs")
                zi = zpool.tile([P, 1], F32, tag="zi", name="zi")
                nc.vector.tensor_add(zs[:], zp[:, 0:1], zp[:, 1:2])
                nc.vector.reciprocal(zi[:], zs[:])
                zinv[(h, ic)] = zi
                nc.vector.tensor_scalar_mul(e_t[:], e_t[:], zi[:])
                nc.sync.dma_start(attn_o[h, ts(ic, P), :], e_t[:])

            def b_side(h, jc, acc):
                # scores^T[j,i] chunk -> exp -> E^T -> ctx delta, accumulated
                # into SBUF on VectorE (PSUM stays transient)
                et_t = etpool.tile([P, S], F32R, tag="et", name="et")
                for ih in range(2):
                    ps = psB.tile([P, 1024], F32, tag="psB", name="pb")
                    for g in range(2):
                        nc.tensor.matmul(
                            ps[:, ts(g, 512)],
                            (ksel(h, jc)),
                            (qgrp(h, ih * 2 + g)),
                            start=True,
                            stop=True,
                        )
                    nc.scalar.activation(
                        et_t[:, ts(ih, 1024)], ps[:], EXP, scale=SCALE
                    )
                dl = psB.tile([P, SC, D], F32, tag="psB", name="dl")
                for ic in range(SC):
                    nc.tensor.matmul(
                        dl[:, ic, :],
                        (et_t[:, ts(ic, P)]),
                        (v_sb[jc][:, h * D : (h + 1) * D]),
                        start=True,
                        stop=True,
                    )
                if jc == 0:
                    nc.vector.tensor_copy(acc[:], dl[:])
                else:
                    nc.vector.tensor_add(acc[:], acc[:], dl[:])

            def ctx_out_step(h, ic, acc):
                # normalize ctx rows by 1/Z, transpose into ctx^T column tiles.
                # Head 2 runs in the kernel tail where ScalarE is idle, so its
                # copies go there; heads 0/1 stay on VectorE (ScalarE is the
                # bottleneck mid-kernel).
                eng = nc.scalar if h == 2 else nc.vector
                cs_t = ctpool.tile([P, D], F32, tag="cs", name="cs")
                if h == 2:
                    nc.scalar.mul(cs_t[:], acc[:, ic, :], zinv[(h, ic)][:])
                else:
                    nc.vector.tensor_scalar(
                        cs_t[:],
                        acc[:, ic, :],
                        zinv[(h, ic)][:],
                        None,
                        op0=mybir.AluOpType.mult,
                    )
                pt = psB.tile([P, 512], F32, tag="psB", name="pt")
                nc.tensor.transpose(pt[0:D, 0:P], cs_t[:], ident[:])
                if h == 2:
                    nc.scalar.copy(ctt[ic][:, h, :], pt[0:D, 0:P])
                else:
                    nc.vector.tensor_copy(ctt[ic][:, h, :], pt[0:D, 0:P])

            def outproj(s):
                pool, tagn = (psA, "psA") if s % 2 == 0 else (psB, "psB")
                po = pool.tile([P, 1024], F32, tag=tagn, name="po")
                o_t = opool.tile([P, HID], F32, tag="osb", name="osb")
                for og in range(2):
                    ow = 512 if og == 0 else 256
                    for h in range(HPC):
                        nc.tensor.matmul(
                            po[:, og * 512 : og * 512 + ow],
                            (ctt[s][:, h, :]),
                            (wo_h[h][:, og * 512 : og * 512 + ow]),
                            start=(h == 0),
                            stop=(h == HPC - 1),
                        )
                nc.scalar.copy(o_t[:], po[:, 0:HID])
                nc.sync.dma_start(out_o[ts(s, P), :], o_t[:])

            # Emission order = per-engine execution order; pair each head's
            # ScalarE-heavy A-side with PE-heavy work from other heads so the
            # bottleneck engine (ScalarE exp) never starves.
            acc0 = accpool.tile([P, SC, D], F32, tag="acc", name="acc0")
            for c in range(CCA):
                nc.sync.dma_start(xcv[c][:], xvT[ts(c, P), :])
            for t in range(SC):
                a_side(0, t)
                v_proj(t)
            xstack.close()
            cpool = ctx.enter_context(tc.tile_pool(name="c", bufs=1))
            # ctx^T as per-s-chunk tiles [d, head, s] (transpose-mode matmuls
            # must write PSUM partition 0, so every head's block lives at 0:64)
            ctt = [cpool.tile([D, HPC, P], F32R, tag=f"ctt{s}", name=f"ctt{s}") for s in range(SC)]

            acc1 = accpool.tile([P, SC, D], F32, tag="acc", name="acc1")
            for t in range(SC):
                a_side(1, t)
                b_side(0, t, acc0)
            for ic in range(SC):
                ctx_out_step(0, ic, acc0)
            acc2 = accpool.tile([P, SC, D], F32, tag="acc", name="acc2")
            for t in range(SC):
                a_side(2, t)
                b_side(1, t, acc1)
                b_side(2, t, acc2)
            for ic in range(SC):
                ctx_out_step(1, ic, acc1)
                ctx_out_step(2, ic, acc2)
                outproj(ic)

    split_sync_waits(nc)
    return nc


_prog = None


def _get_prog():
    global _prog
    if _prog is None:
        _prog = build_program()
    return _prog


def _aug_x(x):
    """[S, HID] activations -> [896, S]: transposed, ones row 768, zero pad."""
    xa = np.zeros((CA, S), np.float32)
    xa[:HID] = np.ascontiguousarray(x.T)
    xa[HID] = 1.0
    return xa


def _aug_w(W, bias, f0, width=F):
    """rows f0:f0+F of W (+bias) -> [896, width]: W_s^T with bias row 768."""
    wa = np.zeros((CA, width), np.float32)
    wa[:HID, :F] = np.ascontiguousarray(W[f0 : f0 + F].T)
    wa[HID, :F] = bias[f0 : f0 + F]
    return wa


def kernel(q, k, v, Wq, bq, Wk, bk, Wv, bv, Wo, bo):
    global LAST_RESULTS
    q, k, v = (np.asarray(a, np.float32) for a in (q, k, v))
    Wq, bq, Wk, bk, Wv, bv, Wo, bo = (
        np.asarray(a, np.float32) for a in (Wq, bq, Wk, bk, Wv, bv, Wo, bo)
    )

    nc = _get_prog()

    xq = [_aug_x(q[b]) for b in range(B)]
    xk = [_aug_x(k[b]) for b in range(B)]
    xv = [_aug_x(v[b]) for b in range(B)]

    in_maps = []
    for core in range(NCORE):
        b, hg = core // 4, core % 4
        f0 = hg * F
        in_maps.append(
            {
                "xqT": xq[b],
                "xkT": xk[b],
                "xvT": xv[b],
                "wqT": _aug_w(Wq, bq, f0),
                "wkT": _aug_w(Wk, bk, f0),
                "wvT": _aug_w(Wv, bv, f0, width=FP),
                "woT": np.ascontiguousarray(Wo[:, f0 : f0 + F].T),
            }
        )

    res = run_bass_kernel_spmd(nc, in_maps, list(range(NCORE)), trace=TRACE)
    LAST_RESULTS = res
    r = res.results

    attn = np.concatenate([r[c]["attn"] for c in range(NCORE)], axis=0).reshape(
        B, NHEAD, S, S
    )
    out = np.stack(
        [
            r[4 * b + 0]["outp"]
            + r[4 * b + 1]["outp"]
            + r[4 * b + 2]["outp"]
            + r[4 * b + 3]["outp"]
            + bo[None, :]
            for b in range(B)
        ]
    ).astype(np.float32)
    return out, attn


# revision 51
# speedup vs baseline: 1.0079x; 1.0079x over previous
"""Trainium2 Bass kernel: multi-head attention (B=2, S=2048, H=768, 12 heads).

Returns (out, attn) like the reference. Sharding: 8 cores, each handles one
(batch, head-group-of-3) pair: core c -> batch c//4, heads 3*(c%4)..3*(c%4)+2.

Per-core plan (dual-orientation attention; every matmul contraction sits on
the partition dim, all operands fp32r for full-rate PE):
  - Host feeds transposed activations/weights; biases are folded in via an
    augmented ones-row (contraction padded to 896 = 7*128 chunks).
  - Projections: Q^T/K^T packed per head-pair, split into per-512-column
    tiles for fine-grained dependency release;
    V natural [2048, 192] (free dim padded to 256 for fp32r full rate).
  - A-side: scores[i,j] per 128-query chunk -> exp on ScalarE (accumulating
    the row-sum Z) -> 1/Z normalize on VectorE -> DMA the attn rows (the
    dominant 403MB output, fully contiguous 8KB rows).
  - B-side: scores^T[j,i] recomputed (cheaper than transposing 12.6M floats
    on-chip) -> exp -> E^T; per-j-chunk ctx deltas land in transient PSUM and
    VectorE accumulates them into SBUF (PSUM stays within 8 banks).
  - ctx normalized by 1/Z while leaving PSUM, transposed per 128x64 block on
    TensorE (transpose outputs must start at PSUM partition 0, hence per-head
    [64,3,128] ctx^T tiles + 3-way-accumulated output projection).
  - Emission order == per-engine execution order, so phases are interleaved
    by hand to keep ScalarE (the bottleneck: 2x 12.6M exps) at ~100% busy:
    head0-A pairs with the V projection, head(h)-A with head(h-1)-B.
  - Host gathers: attn concat, partial out summed across head-group cores.
Cost-model timeline: ~361us/core; engines ACT ~239us, DMA ~226us, PE ~225us
(balanced within 6%).
Startup is DMA-bound (x loads pipelined via 10 shared chunk slots); the
steady state holds ScalarE at ~100%; tail copies ride the then-idle ScalarE.
"""

import sys

sys.path.insert(0, "/opt/trn_rl_repo")

import numpy as np
from concourse import bass, masks, mybir, tile
from concourse.bass_utils import run_bass_kernel_spmd

F32 = mybir.dt.float32
F32R = mybir.dt.float32r
EXP = mybir.ActivationFunctionType.Exp

B, S, HID = 2, 2048, 768
NHEAD, D = 12, 64
NCORE = 8
HPC = 3            # heads per core
F = HPC * D        # 192 projected features per core
P = 128
SC = S // P        # 16 s-chunks
CCA = 7            # augmented contraction chunks (896 = 768 feat + bias row + pad)
CA = CCA * P       # 896
FP = 256           # V-projection free dim padded to 256 (fp32r full-rate needs >=256)
SCALE = 1.0 / float(np.sqrt(D))

ts = bass.ts

# toggles for test harness
TRACE = False
LAST_RESULTS = None


def split_sync_waits(nc, limit=1):
    """walrus codegen rejects CTRL instructions (the kernel-tail Drain)
    carrying more sem waits than it has slots; move excess waits onto
    preceding same-engine nops."""
    ctr = 0
    for f in nc.m.functions:
        for bb in f.blocks:
            out = []
            changed = False
            for inst in bb.instructions:
                si = inst.sync_info
                if si is not None and si.on_wait and len(si.on_wait) > limit:
                    waits = list(si.on_wait)
                    head, rest = waits[:limit], waits[limit:]
                    for i in range(0, len(rest), limit):
                        nop = mybir.InstNoOp(name=f"wsplit{ctr}", ins=[], outs=[])
                        ctr += 1
                        nop.engine = inst.engine
                        nop.sync_info = mybir.SyncInfo(
                            on_wait=rest[i : i + limit], on_update=[]
                        )
                        out.append(nop)
                    si.on_wait = head
                    changed = True
                out.append(inst)
            if changed:
                bb.instructions = out
    return ctr


def build_program():
    nc = bass.Bass()

    xqT = nc.declare_dram_parameter("xqT", [CA, S], F32R, isOutput=False)
    xkT = nc.declare_dram_parameter("xkT", [CA, S], F32R, isOutput=False)
    xvT = nc.declare_dram_parameter("xvT", [CA, S], F32R, isOutput=False)
    wqT = nc.declare_dram_parameter("wqT", [CA, F], F32R, isOutput=False)
    wkT = nc.declare_dram_parameter("wkT", [CA, F], F32R, isOutput=False)
    wvT = nc.declare_dram_parameter("wvT", [CA, FP], F32R, isOutput=False)
    woT = nc.declare_dram_parameter("woT", [F, HID], F32R, isOutput=False)
    attn_o = nc.declare_dram_parameter("attn", [HPC, S, S], F32, isOutput=True)
    out_o = nc.declare_dram_parameter("outp", [S, HID], F32, isOutput=True)

    from contextlib import ExitStack

    with tile.TileContext(nc) as tc:
        with ExitStack() as ctx:
            wpool = ctx.enter_context(tc.tile_pool(name="w", bufs=1))
            qkpool = ctx.enter_context(tc.tile_pool(name="qk", bufs=1))
            vpool = ctx.enter_context(tc.tile_pool(name="v", bufs=1))
            epool = ctx.enter_context(tc.tile_pool(name="e", bufs=2))
            etpool = ctx.enter_context(tc.tile_pool(name="et", bufs=2))
            stat = ctx.enter_context(tc.tile_pool(name="st", bufs=4))
            zpool = ctx.enter_context(tc.tile_pool(name="z", bufs=34))
            ctpool = ctx.enter_context(tc.tile_pool(name="ct", bufs=4))
            accpool = ctx.enter_context(tc.tile_pool(name="acc", bufs=2))
            opool = ctx.enter_context(tc.tile_pool(name="o", bufs=2))
            # PSUM (8 banks): psA slot [128,1024] x2 bufs = 4 banks (scores-A,
            # QK-proj, even outproj), psB slot [128,1024] x2 bufs = 4 banks
            # (scores-B, V-proj, ctx deltas, transposes, odd outproj).
            psA = ctx.enter_context(tc.tile_pool(name="psA", bufs=2, space="PSUM"))
            psB = ctx.enter_context(tc.tile_pool(name="psB", bufs=2, space="PSUM"))

            ident = wpool.tile([P, P], F32, tag="ident")
            masks.make_identity(nc, ident[:])

            # ---- weights ----
            # x chunks + projection weight chunks live in a scoped pool that
            # closes after phase 1, freeing ~72KB/partition for the ctx pool
            xstack = ExitStack()
            xpool = xstack.enter_context(tc.tile_pool(name="x", bufs=CCA + 3))
            # one batched DMA per weight tensor (24 small DMAs would each pay
            # the fixed per-transfer overhead before the x loads can start)
            wq_a = xpool.tile([P, CCA, F], F32R, tag="wqa", bufs=1)
            wk_a = xpool.tile([P, CCA, F], F32R, tag="wka", bufs=1)
            wv_a = xpool.tile([P, CCA, FP], F32R, tag="wva", bufs=1)
            nc.sync.dma_start(wq_a[:], wqT.rearrange("(c p) f -> p c f", p=P))
            nc.sync.dma_start(wk_a[:], wkT.rearrange("(c p) f -> p c f", p=P))
            nc.sync.dma_start(wv_a[:], wvT.rearrange("(c p) f -> p c f", p=P))
            wq_c = [wq_a[:, c, :] for c in range(CCA)]
            wk_c = [wk_a[:, c, :] for c in range(CCA)]
            wv_c = [wv_a[:, c, :] for c in range(CCA)]
            wo_a = wpool.tile([D, HPC, HID], F32R, tag="woa")
            nc.sync.dma_start(wo_a[:], woT.rearrange("(h d) o -> d h o", d=D))
            wo_h = [wo_a[:, h, :] for h in range(HPC)]

            # ---- Q/K projections ----
            # per-512-column tiles: finer dependency granularity lets the
            # first exps start as soon as their column group is projected
            qtn = [qkpool.tile([P, 512], F32R, tag=f"qtn{n}", name=f"qtn{n}") for n in range(4)]
            qbn = [qkpool.tile([D, 512], F32R, tag=f"qbn{n}", name=f"qbn{n}") for n in range(4)]
            ktn = [qkpool.tile([P, 512], F32R, tag=f"ktn{n}", name=f"ktn{n}") for n in range(4)]
            kbn = [qkpool.tile([D, 512], F32R, tag=f"kbn{n}", name=f"kbn{n}") for n in range(4)]
            v_sb = [vpool.tile([P, F], F32R, tag=f"v{s}", name=f"v{s}") for s in range(SC)]

            def _grp(ta, tb, h, g):
                t = ta[g] if h < 2 else tb[g]
                r0 = D * (h % 2) if h < 2 else 0
                return t[r0 : r0 + D, :]

            def _sel(ta, tb, h, c):
                t = ta[c // 4] if h < 2 else tb[c // 4]
                r0 = D * (h % 2) if h < 2 else 0
                o = (c % 4) * P
                return t[r0 : r0 + D, o : o + P]

            def qgrp(h, g):
                return _grp(qtn, qbn, h, g)

            def qsel(h, c):
                return _sel(qtn, qbn, h, c)

            def kgrp(h, g):
                return _grp(ktn, kbn, h, g)

            def ksel(h, c):
                return _sel(ktn, kbn, h, c)

            for xdram, w_c, dst01, dst2 in (
                (xqT, wq_c, qtn, qbn),
                (xkT, wk_c, ktn, kbn),
            ):
                xc = [xpool.tile([P, S], F32R, tag="xc", name="xc") for _ in range(CCA)]
                for c in range(CCA):
                    nc.sync.dma_start(xc[c][:], xdram[ts(c, P), :])
                for m in range(2):  # f rows 0:128 / 128:192
                    mp = P if m == 0 else F - P
                    for n in range(4):
                        ps = psB.tile([P, 512], F32, tag="psB", name="pj")
                        for c in range(CCA):
                            nc.tensor.matmul(
                                ps[:mp, :],
                                (w_c[c][:, m * P : m * P + mp]),
                                (xc[c][:, ts(n, 512)]),
                                start=(c == 0),
                                stop=(c == CCA - 1),
                            )
                        if m == 0:
                            nc.vector.tensor_copy(dst01[n][:], ps[:])
                        else:
                            nc.vector.tensor_copy(dst2[n][:], ps[0:D, :])

            xcv = [xpool.tile([P, S], F32R, tag="xc", name="xc") for _ in range(CCA - 1)]
            xcvb = xpool.tile([1, S], F32R, tag="xcb", name="xcvb", bufs=1)

            def v_proj(s):
                ps = psB.tile([P, 512], F32, tag="psB", name="pv")
                for c in range(CCA):
                    lhsT = (
                        xcv[c][:, ts(s, P)] if c < CCA - 1 else xcvb[:, ts(s, P)]
                    )
                    rhs = wv_c[c][:] if c < CCA - 1 else wv_c[c][0:1, :]
                    nc.tensor.matmul(
                        ps[:, :FP],
                        lhsT,
                        rhs,
                        start=(c == 0),
                        stop=(c == CCA - 1),
                    )
                nc.vector.tensor_copy(v_sb[s][:], ps[:, :F])

            # ---- attention ----
            zinv = {}
            def a_side(h, ic):
                # scores[i,j] for one 128-query chunk; softmax row pass
                e_t = epool.tile([P, S], F32, tag="e", name="e")
                zp = stat.tile([P, 2], F32, tag="zp", name="zp")
                for jh in range(2):
                    ps = psA.tile([P, 1024], F32, tag="psA", name="pa")
                    for g in range(2):
                        nc.tensor.matmul(
                            ps[:, ts(g, 512)],
                            (qsel(h, ic)),
                            (kgrp(h, jh * 2 + g)),
                            start=True,
                            stop=True,
                        )
                    nc.scalar.activation(
                        e_t[:, ts(jh, 1024)], ps[:], EXP,
                        scale=SCALE, accum_out=zp[:, jh : jh + 1],
                    )
                zs = stat.tile([P, 1], F32, tag="zs", name="zs")
                zi = zpool.tile([P, 1], F32, tag="zi", name="zi")
                nc.vector.tensor_add(zs[:], zp[:, 0:1], zp[:, 1:2])
                nc.vector.reciprocal(zi[:], zs[:])
                zinv[(h, ic)] = zi
                nc.vector.tensor_scalar_mul(e_t[:], e_t[:], zi[:])
                nc.sync.dma_start(attn_o[h, ts(ic, P), :], e_t[:])

            def b_side(h, jc, acc):
                # scores^T[j,i] chunk -> exp -> E^T -> ctx delta, accumulated
                # into SBUF on VectorE (PSUM stays transient)
                et_t = etpool.tile([P, S], F32R, tag="et", name="et")
                for ih in range(2):
                    ps = psB.tile([P, 1024], F32, tag="psB", name="pb")
                    for g in range(2):
                        nc.tensor.matmul(
                            ps[:, ts(g, 512)],
                            (ksel(h, jc)),
                            (qgrp(h, ih * 2 + g)),
                            start=True,
                            stop=True,
                        )
                    nc.scalar.activation(
                        et_t[:, ts(ih, 1024)], ps[:], EXP, scale=SCALE
                    )
                dl = psB.tile([P, SC, D], F32, tag="psB", name="dl")
                for ic in range(SC):
                    nc.tensor.matmul(
                        dl[:, ic, :],
                        (et_t[:, ts(ic, P)]),
                        (v_sb[jc][:, h * D : (h + 1) * D]),
                        start=True,
                        stop=True,
                    )
                if jc == 0:
                    nc.vector.tensor_copy(acc[:], dl[:])
                else:
                    nc.vector.tensor_add(acc[:], acc[:], dl[:])

            def ctx_out_step(h, ic, acc):
                # normalize ctx rows by 1/Z, transpose into ctx^T column tiles.
                # Head 2 runs in the kernel tail where ScalarE is idle, so its
                # copies go there; heads 0/1 stay on VectorE (ScalarE is the
                # bottleneck mid-kernel).
                eng = nc.scalar if h == 2 else nc.vector
                cs_t = ctpool.tile([P, D], F32, tag="cs", name="cs")
                if h == 2:
                    nc.scalar.mul(cs_t[:], acc[:, ic, :], zinv[(h, ic)][:])
                else:
                    nc.vector.tensor_scalar(
                        cs_t[:],
                        acc[:, ic, :],
                        zinv[(h, ic)][:],
                        None,
                        op0=mybir.AluOpType.mult,
                    )
                pt = psB.tile([P, 512], F32, tag="psB", name="pt")
                nc.tensor.transpose(pt[0:D, 0:P], cs_t[:], ident[:])
                if h == 2:
                    nc.scalar.copy(ctt[ic][:, h, :], pt[0:D, 0:P])
                else:
                    nc.vector.tensor_copy(ctt[ic][:, h, :], pt[0:D, 0:P])

            def outproj(s):
                pool, tagn = (psA, "psA") if s % 2 == 0 else (psB, "psB")
                po = pool.tile([P, 1024], F32, tag=tagn, name="po")
                o_t = opool.tile([P, HID], F32, tag="osb", name="osb")
                for og in range(2):
                    ow = 512 if og == 0 else 256
                    for h in range(HPC):
                        nc.tensor.matmul(
                            po[:, og * 512 : og * 512 + ow],
                            (ctt[s][:, h, :]),
                            (wo_h[h][:, og * 512 : og * 512 + ow]),
                            start=(h == 0),
                            stop=(h == HPC - 1),
                        )
                nc.scalar.copy(o_t[:], po[:, 0:HID])
                nc.sync.dma_start(out_o[ts(s, P), :], o_t[:])

            # Emission order = per-engine execution order; pair each head's
            # ScalarE-heavy A-side with PE-heavy work from other heads so the
            # bottleneck engine (ScalarE exp) never starves.
            acc0 = accpool.tile([P, SC, D], F32, tag="acc", name="acc0")
            for c in range(CCA - 1):
                nc.sync.dma_start(xcv[c][:], xvT[ts(c, P), :])
            nc.sync.dma_start(xcvb[:], xvT[HID : HID + 1, :])
            for t in range(SC):
                a_side(0, t)
                v_proj(t)
            xstack.close()
            cpool = ctx.enter_context(tc.tile_pool(name="c", bufs=1))
            # ctx^T as per-s-chunk tiles [d, head, s] (transpose-mode matmuls
            # must write PSUM partition 0, so every head's block lives at 0:64)
            ctt = [cpool.tile([D, HPC, P], F32R, tag=f"ctt{s}", name=f"ctt{s}") for s in range(SC)]

            acc1 = accpool.tile([P, SC, D], F32, tag="acc", name="acc1")
            for t in range(SC):
                a_side(1, t)
                b_side(0, t, acc0)
            for ic in range(SC):
                ctx_out_step(0, ic, acc0)
            acc2 = accpool.tile([P, SC, D], F32, tag="acc", name="acc2")
            for t in range(SC):
                a_side(2, t)
                b_side(1, t, acc1)
                b_side(2, t, acc2)
            for ic in range(SC):
                ctx_out_step(1, ic, acc1)
                ctx_out_step(2, ic, acc2)
                outproj(ic)

    split_sync_waits(nc)
    return nc


_prog = None


def _get_prog():
    global _prog
    if _prog is None:
        _prog = build_program()
    return _prog


def _aug_x(x):
    """[S, HID] activations -> [896, S]: transposed, ones row 768, zero pad."""
    xa = np.zeros((CA, S), np.float32)
    xa[:HID] = np.ascontiguousarray(x.T)
    xa[HID] = 1.0
    return xa


def _aug_w(W, bias, f0, width=F):
    """rows f0:f0+F of W (+bias) -> [896, width]: W_s^T with bias row 768."""
    wa = np.zeros((CA, width), np.float32)
    wa[:HID, :F] = np.ascontiguousarray(W[f0 : f0 + F].T)
    wa[HID, :F] = bias[f0 : f0 + F]
    return wa


def kernel(q, k, v, Wq, bq, Wk, bk, Wv, bv, Wo, bo):
    global LAST_RESULTS
    q, k, v = (np.asarray(a, np.float32) for a in (q, k, v))
    Wq, bq, Wk, bk, Wv, bv, Wo, bo = (
        np.asarray(a, np.float32) for a in (Wq, bq, Wk, bk, Wv, bv, Wo, bo)
    )

    nc = _get_prog()

    xq = [_aug_x(q[b]) for b in range(B)]
    xk = [_aug_x(k[b]) for b in range(B)]
    xv = [_aug_x(v[b]) for b in range(B)]

    in_maps = []
    for core in range(NCORE):
        b, hg = core // 4, core % 4
        f0 = hg * F
        in_maps.append(
            {
                "xqT": xq[b],
                "xkT": xk[b],
                "xvT": xv[b],
                "wqT": _aug_w(Wq, bq, f0),
                "wkT": _aug_w(Wk, bk, f0),
                "wvT": _aug_w(Wv, bv, f0, width=FP),
                "woT": np.ascontiguousarray(Wo[:, f0 : f0 + F].T),
            }
        )

    res = run_bass_kernel_spmd(nc, in_maps, list(range(NCORE)), trace=TRACE)
    LAST_RESULTS = res
    r = res.results

    attn = np.concatenate([r[c]["attn"] for c in range(NCORE)], axis=0).reshape(
        B, NHEAD, S, S
    )
    out = np.stack(
        [
            r[4 * b + 0]["outp"]
            + r[4 * b + 1]["outp"]
            + r[4 * b + 2]["outp"]
            + r[4 * b + 3]["outp"]
            + bo[None, :]
            for b in range(B)
        ]
    ).astype(np.float32)
    return out, attn
